# revision 20
# baseline (speedup 1.0000x reference)
"""GAT layer kernel for Trainium2 (8 NeuronCores, SPMD data-parallel over B).

Reference computation (per (b,t) slice, N=512 nodes, D=F=128):
    h = x_bt @ W
    e[i,j] = leaky_relu(e_src[i] + e_dst[j], 0.2)
    e masked by adj|I, row-softmax, out = elu(alpha @ h)

v5 dataflow per (b,t), in eT = e^T [j, i] orientation so aggregation runs
as PSUM-accumulated matmuls with j as contraction dim. All ops obey the
BIR legality rules: gpsimd is SBUF-only, TT has at most one PSUM input,
stt/reciprocal are SBUF-only, matmul operand dtypes match.

    xT   : transpose-DMA loads x straight from DRAM as bf16 [d, n] --
           no PE transposes, no PSUM evacuation for x at all.
    ev   : [es; ed] = WSD.T @ xT (one matmul); two DVE tensor_scalar
           fixups build ev_rhs=[es;1], ev_lhs=[1;ed] (f32r SBUF)
    e    : rank-2 matmuls, 2-chunk halves     [j, i] PSUM (ring 2)
    z1   : Prelu halves on ACT -> z bf16 SBUF; Exp in-place (one ACT op)
    h    : xT chunks @ W (bf16) -> PSUM -> DVE evac bf16
    mask : z2 = z * M01 (0/1 bf16), split DVE/Pool by chunk
    s    : ones.T @ z2 accumulated            [1, i] PSUM
    uT   : h.T @ z2 accumulated               [f, i] PSUM
    r    : s -> SBUF (DVE), PE-transpose to columns, reciprocal [128,4]
    v    : uT -> SBUF bf16 (DVE), PE-transpose back to [i, f] chunks,
           v = u * r per chunk (DVE tensor_scalar, per-partition ptr)
    out  : elu(v) = max(v, min(exp(v)-1, 0)): ACT exp, Pool e1, Pool max
"""

import numpy as np

B, N, T, D, F = 16, 512, 12, 128, 128
NCORES = 8
B_PER_CORE = B // NCORES
NCH = N // 128  # 4 chunks of 128 nodes


def _build_program(reps=1):
    import concourse.bacc as bacc
    import concourse.tile as tile
    from concourse import mybir

    import os
    F32 = mybir.dt.float32
    F32R = mybir.dt.float32r
    BF16 = mybir.dt.bfloat16
    AF = mybir.ActivationFunctionType
    ALU = mybir.AluOpType

    nc = bacc.Bacc()

    def eng(name):
        return {"pool": nc.gpsimd, "vector": nc.vector}[name]

    # engine assignment knobs
    E_HCOPY = eng(os.environ.get("K_HCOPY", "vector"))
    E_SROW = eng(os.environ.get("K_SROW", "vector"))
    MSPLIT = int(os.environ.get("K_MSPLIT", "1"))  # chunks on DVE; rest Pool
    E_E1 = eng(os.environ.get("K_E1", "pool"))
    TAIL = os.environ.get("K_TAIL", "vone")  # mchunk|vone|dvemax
    MASKMODE = os.environ.get("K_MASKMODE", "prefill")  # prefill|mult_after
    E_OMAX = eng(os.environ.get("K_OMAX", "pool"))

    x_h = nc.declare_dram_parameter("x", [B_PER_CORE, N, T, D], BF16, isOutput=False)
    wb_h = nc.declare_dram_parameter("wb", [D, F], BF16, isOutput=False)
    wsd_h = nc.declare_dram_parameter("wsd", [D, 2], BF16, isOutput=False)
    capt_h = nc.declare_dram_parameter("capt", [NCH, 128, N], BF16, isOutput=False)
    maskt_h = nc.declare_dram_parameter("maskt", [NCH, 128, N], BF16, isOutput=False)
    sel_h = nc.declare_dram_parameter("sel", [2, 2], F32, isOutput=False)
    ident_h = nc.declare_dram_parameter("ident", [128, 128], F32, isOutput=False)
    identb_h = nc.declare_dram_parameter("identb", [128, 128], BF16, isOutput=False)
    ones_h = nc.declare_dram_parameter("onescol", [128, 1], BF16, isOutput=False)
    out_h = nc.declare_dram_parameter("out", [B_PER_CORE, N, T, F], F32, isOutput=True)

    NBT = B_PER_CORE * T

    with tile.TileContext(nc) as tc:
        with (
            tc.tile_pool(name="consts", bufs=1) as consts,
            tc.tile_pool(name="xbuf", bufs=1) as xbuf,
            tc.tile_pool(name="work", bufs=int(os.environ.get("K_WORK", "5"))) as work,
            tc.tile_pool(name="zpool", bufs=int(os.environ.get("K_BIG", "5"))) as zpool,
            tc.tile_pool(name="z2pool", bufs=int(os.environ.get("K_BIG2", "5"))) as z2pool,
            tc.tile_pool(name="hpool", bufs=int(os.environ.get("K_HP", "5"))) as hpool,
            # PSUM (16KB/partition): mm{ev,h,s,v} ring3 = 6KB,
            # eadd halves [128,2,512] ring2 = 8KB, ut{uT,sc} ring1 = 2KB
            tc.tile_pool(name="mm_ps", bufs=int(os.environ.get("K_MM", "3")), space="PSUM") as mm_ps,
            tc.tile_pool(name="eadd_ps", bufs=int(os.environ.get("K_EADD", "2")), space="PSUM") as eadd_ps,
            tc.tile_pool(name="ut_ps", bufs=int(os.environ.get("K_UT", "1")), space="PSUM") as ut_psp,
        ):
            wb_sb = consts.tile([D, F], BF16)
            wsd_sb = consts.tile([D, 2], BF16)
            cap_sb = consts.tile([128, NCH, N], BF16)
            mask_sb = consts.tile([128, NCH, N], BF16)
            sel_sb = consts.tile([2, 2], F32)
            id_sb = consts.tile([128, 128], F32)
            idb_sb = consts.tile([128, 128], BF16)
            ones_sb = consts.tile([128, 1], BF16)
            nc.sync.dma_start(out=wb_sb, in_=wb_h[:, :])
            nc.sync.dma_start(out=wsd_sb, in_=wsd_h[:, :])
            for c in range(NCH):
                nc.sync.dma_start(out=cap_sb[:, c, :], in_=capt_h[c, :, :])
                nc.sync.dma_start(out=mask_sb[:, c, :], in_=maskt_h[c, :, :])
            nc.sync.dma_start(out=sel_sb, in_=sel_h[:, :])
            nc.sync.dma_start(out=id_sb, in_=ident_h[:, :])
            nc.sync.dma_start(out=idb_sb, in_=identb_h[:, :])
            nc.sync.dma_start(out=ones_sb, in_=ones_h[:, :])

            # ---- transpose-DMA all of x: [n, d] slices land as [d, n] bf16.
            #      Chunked [128, 128] so engines spread and bt 0 starts early.
            xT_all = xbuf.tile([128, NBT, N], BF16, tag="xT")
            for b in range(B_PER_CORE):
                for t in range(T):
                    k = b * T + t
                    for c in range(NCH):
                        nc.sync.dma_start_transpose(
                            out=xT_all[:, k, c * 128:(c + 1) * 128],
                            in_=x_h[b, c * 128:(c + 1) * 128, t, :])

            # persistent ev operand rings: row1 of ev_rhs and row0 of
            # ev_lhs hold constant 1.0 (memset once); per-bt one DVE
            # evacuate + two SBUF-to-SBUF row DMAs fill in es/ed.
            EVFIX = os.environ.get("K_EVFIX", "engines")
            EVN = int(os.environ.get("K_EVN", "4"))
            ev_rhs_ring = []
            ev_lhs_ring = []
            if EVFIX == "dma":
                for i in range(EVN):
                    er = work.tile([2, N], F32R, tag="ev_rhs", name=f"evr{i}")
                    el = work.tile([2, N], F32R, tag="ev_lhs", name=f"evl{i}")
                    nc.vector.memset(er[1:2, :], 1.0)
                    nc.vector.memset(el[0:1, :], 1.0)
                    ev_rhs_ring.append(er)
                    ev_lhs_ring.append(el)

            st = [dict() for _ in range(NBT)]

            def stage1(k):
                xT = xT_all[:, k, :]
                # ev rows [es; ed] -> ev_rhs = [es; ones], ev_lhs = [ones; ed]
                ev_ps = mm_ps.tile([2, N], F32, tag="mm")
                nc.tensor.matmul(ev_ps, wsd_sb, xT, start=True, stop=True)
                if EVFIX == "dma":
                    evb_sb = work.tile([2, N], F32R, tag="evb")
                    nc.vector.tensor_copy(out=evb_sb, in_=ev_ps)
                    ev_rhs = ev_rhs_ring[k % EVN]
                    ev_lhs = ev_lhs_ring[k % EVN]
                    nc.sync.dma_start(out=ev_rhs[0:1, :], in_=evb_sb[0:1, :])
                    nc.sync.dma_start(out=ev_lhs[1:2, :], in_=evb_sb[1:2, :])
                else:
                    ev_rhs = work.tile([2, N], F32R, tag="ev_rhs")
                    ev_lhs = work.tile([2, N], F32R, tag="ev_lhs")
                    nc.vector.tensor_scalar(
                        out=ev_rhs, in0=ev_ps, scalar1=sel_sb[:, 0:1],
                        scalar2=sel_sb[:, 1:2], op0=ALU.mult, op1=ALU.add)
                    nc.vector.tensor_scalar(
                        out=ev_lhs, in0=ev_ps, scalar1=sel_sb[:, 1:2],
                        scalar2=sel_sb[:, 0:1], op0=ALU.mult, op1=ALU.add)

                # rank-2 logits by halves -> Prelu -> z bf16. In prefill
                # mode the mask bias (0/-1e9) is accumulated into PSUM by an
                # identity matmul before the rank-2, so no post-exp multiply.
                z_sb = zpool.tile([128, NCH, N], BF16, tag="z_sb")
                for ha in range(2):
                    e_ps = eadd_ps.tile([128, 2, N], F32, tag="eadd")
                    for ci in range(2):
                        c = 2 * ha + ci
                        if MASKMODE == "prefill":
                            nc.tensor.matmul(
                                e_ps[:, ci, :], idb_sb, mask_sb[:, c, :],
                                start=True, stop=False)
                        nc.tensor.matmul(
                            e_ps[:, ci, :],
                            ev_lhs[:, c * 128:(c + 1) * 128],
                            ev_rhs, start=(MASKMODE != "prefill"), stop=True)
                    nc.scalar.activation(
                        z_sb[:, 2 * ha:2 * ha + 2, :], e_ps,
                        AF.Prelu, alpha=0.2)

                # h chunks [j, f] bf16
                h_ps = mm_ps.tile([128, NCH, F], F32, tag="mm")
                for c in range(NCH):
                    nc.tensor.matmul(h_ps[:, c, :], xT[:, c * 128:(c + 1) * 128],
                                     wb_sb, start=True, stop=True)
                h_sb = hpool.tile([128, NCH, F], BF16, tag="h_sb")
                E_HCOPY.tensor_copy(out=h_sb, in_=h_ps)

                # exp in place; mult_after additionally masks into z2
                nc.scalar.activation(z_sb, z_sb, AF.Exp)
                if MASKMODE == "prefill":
                    z2_sb = z_sb
                else:
                    z2_sb = z2pool.tile([128, NCH, N], BF16, tag="z2_sb")
                    m = MSPLIT
                    if m > 0:
                        nc.vector.tensor_tensor(
                            out=z2_sb[:, 0:m, :], in0=z_sb[:, 0:m, :],
                            in1=cap_sb[:, 0:m, :], op=ALU.mult)
                    if m < NCH:
                        nc.gpsimd.tensor_tensor(
                            out=z2_sb[:, m:NCH, :], in0=z_sb[:, m:NCH, :],
                            in1=cap_sb[:, m:NCH, :], op=ALU.mult)
                st[k]["h_sb"] = h_sb
                st[k]["z2_sb"] = z2_sb

            def stage2(k):
                h_sb, z2_sb = st[k]["h_sb"], st[k]["z2_sb"]
                s_ps = mm_ps.tile([1, N], F32, tag="mm")
                for c in range(NCH):
                    nc.tensor.matmul(s_ps, ones_sb, z2_sb[:, c, :],
                                     start=(c == 0), stop=(c == NCH - 1))
                uT_ps = ut_psp.tile([128, N], F32, tag="ut")
                for c in range(NCH):
                    nc.tensor.matmul(uT_ps, h_sb[:, c, :], z2_sb[:, c, :],
                                     start=(c == 0), stop=(c == NCH - 1))

                s_row = work.tile([1, N], F32, tag="s_row")
                E_SROW.tensor_copy(out=s_row, in_=s_ps)
                uT_sb = work.tile([128, N], BF16, tag="uT_sb")
                nc.vector.tensor_copy(out=uT_sb, in_=uT_ps)
                sc_ps = ut_psp.tile([128, NCH], F32, tag="ut")
                for c in range(NCH):
                    nc.tensor.transpose(
                        sc_ps[:, c:c + 1], s_row[0:1, c * 128:(c + 1) * 128],
                        id_sb[0:1, 0:1])
                sc_sb = work.tile([128, NCH], F32, tag="sc_sb")
                nc.vector.tensor_copy(out=sc_sb, in_=sc_ps)
                r_cols = work.tile([128, NCH], F32, tag="r_cols")
                nc.vector.reciprocal_approx_fast(r_cols, sc_sb)
                st[k]["uT_sb"] = uT_sb
                st[k]["r_cols"] = r_cols

            def stage3(k):
                b, t = divmod(k, T)
                uT_sb, r_cols = st[k]["uT_sb"], st[k]["r_cols"]
                u_ps = mm_ps.tile([128, NCH, F], BF16, tag="mm")
                for c in range(NCH):
                    nc.tensor.transpose(
                        u_ps[:, c, :], uT_sb[:, c * 128:(c + 1) * 128], idb_sb)
                # elu(v) = relu(v) + min(exp(v)-1, 0) = max(v, min(exp(v)-1, 0))
                if TAIL == "mchunk":
                    # m = max(u*r,0) DVE per chunk; t = exp(u*r) ACT per chunk
                    m_sb = work.tile([128, NCH, F], F32, tag="m_sb")
                    t_sb = work.tile([128, NCH, F], F32, tag="t_sb")
                    for c in range(NCH):
                        nc.vector.tensor_scalar(
                            out=m_sb[:, c, :], in0=u_ps[:, c, :],
                            scalar1=r_cols[:, c:c + 1], scalar2=0.0,
                            op0=ALU.mult, op1=ALU.max)
                        nc.scalar.activation(
                            t_sb[:, c, :], u_ps[:, c, :], AF.Exp,
                            scale=r_cols[:, c:c + 1])
                    e1_sb = work.tile([128, NCH, F], F32, tag="e1_sb")
                    E_E1.tensor_scalar(
                        out=e1_sb, in0=t_sb, scalar1=-1.0, scalar2=0.0,
                        op0=ALU.add, op1=ALU.min)
                    o_sb = work.tile([128, NCH, F], F32, tag="o_sb")
                    E_OMAX.tensor_tensor(
                        out=o_sb, in0=m_sb, in1=e1_sb, op=ALU.add)
                else:
                    # v materialized by DVE; one ACT exp; m on Pool ts;
                    # tail == "vone": o = m + e1 (Pool); "dvemax": o = DVE max
                    v_sb = work.tile([128, NCH, F], F32, tag="v_sb")
                    for c in range(NCH):
                        nc.vector.tensor_scalar(
                            out=v_sb[:, c, :], in0=u_ps[:, c, :],
                            scalar1=r_cols[:, c:c + 1], scalar2=None,
                            op0=ALU.mult)
                    t_sb = work.tile([128, NCH, F], F32, tag="t_sb")
                    nc.scalar.activation(t_sb, v_sb, AF.Exp)
                    e1_sb = work.tile([128, NCH, F], F32, tag="e1_sb")
                    E_E1.tensor_scalar(
                        out=e1_sb, in0=t_sb, scalar1=-1.0, scalar2=0.0,
                        op0=ALU.add, op1=ALU.min)
                    o_sb = work.tile([128, NCH, F], F32, tag="o_sb")
                    if TAIL == "dvemax":
                        nc.vector.tensor_tensor(
                            out=o_sb, in0=v_sb, in1=e1_sb, op=ALU.max)
                    else:
                        m_sb = work.tile([128, NCH, F], F32, tag="m_sb")
                        nc.gpsimd.tensor_scalar(
                            out=m_sb, in0=v_sb, scalar1=0.0, scalar2=None,
                            op0=ALU.max)
                        E_OMAX.tensor_tensor(
                            out=o_sb, in0=m_sb, in1=e1_sb, op=ALU.add)
                o_dst = out_h[b, :, t, :].rearrange("(c p) f -> p c f", p=128)
                nc.sync.dma_start(out=o_dst, in_=o_sb)
                st[k].clear()

            # software-pipelined emission with stage lag
            LAG = int(os.environ.get("K_LAG", "3"))

            ORDER = os.environ.get("K_ORDER", "312")

            def body(_iv=None, unroll=1):
                for k in range(NBT + 2 * LAG):
                    def s1():
                        if k < NBT:
                            stage1(k)
                    def s2():
                        if LAG <= k < NBT + LAG:
                            stage2(k - LAG)
                    def s3():
                        if k >= 2 * LAG:
                            stage3(k - 2 * LAG)
                    for ch in ORDER:
                        {"1": s1, "2": s2, "3": s3}[ch]()

            if reps == 1:
                body()
            else:
                with tc.For_i(0, reps, 1) as _iv:
                    body(_iv)

    nc.finalize()
    return nc


def prepare(x, W, a_src, a_dst, adj):
    """Build the program + per-core input maps (shared by kernel() and bench)."""
    import ml_dtypes

    bf16 = ml_dtypes.bfloat16
    x = np.ascontiguousarray(x, dtype=np.float32)
    W = np.ascontiguousarray(W, dtype=np.float32)
    a_src = np.asarray(a_src, dtype=np.float32)
    a_dst = np.asarray(a_dst, dtype=np.float32)
    adj = np.asarray(adj)

    allowed = (adj > 0) | np.eye(N, dtype=bool)               # [i, j]
    m01 = np.where(allowed, 1.0, 0.0).astype(np.float32)      # [i, j]
    capt = np.ascontiguousarray(m01.T.reshape(NCH, 128, N)).astype(bf16)
    mneg = np.where(allowed, 0.0, -1e9).astype(np.float32)
    maskt = np.ascontiguousarray(mneg.T.reshape(NCH, 128, N)).astype(bf16)
    ws = W @ a_src
    wd = W @ a_dst
    wsd = np.stack([ws, wd], axis=1).astype(bf16)
    sel = np.array([[1.0, 0.0], [0.0, 1.0]], dtype=np.float32)
    ident = np.eye(128, dtype=np.float32)
    onescol = np.ones((128, 1), dtype=np.float32).astype(bf16)

    nc = _build_program()

    in_maps = []
    for c in range(NCORES):
        in_maps.append({
            "x": np.ascontiguousarray(x[c * B_PER_CORE:(c + 1) * B_PER_CORE]).astype(bf16),
            "wb": W.astype(bf16), "wsd": wsd, "capt": capt, "sel": sel,
            "ident": ident, "identb": ident.astype(bf16), "onescol": onescol,
            "maskt": maskt,
        })
    return nc, in_maps


def kernel(x, W, a_src, a_dst, adj):
    from concourse.bass_utils import run_bass_kernel_spmd

    nc, in_maps = prepare(x, W, a_src, a_dst, adj)
    res = run_bass_kernel_spmd(nc, in_maps, list(range(NCORES)))
    out = np.concatenate([res.results[c]["out"] for c in range(NCORES)], axis=0)
    return out  # [B, N, T, F]


# revision 27
# speedup vs baseline: 1.0370x; 1.0370x over previous
"""GAT layer kernel for Trainium2 (8 NeuronCores, SPMD data-parallel over B).

Reference computation (per (b,t) slice, N=512 nodes, D=F=128):
    h = x_bt @ W
    e[i,j] = leaky_relu(e_src[i] + e_dst[j], 0.2)
    e masked by adj|I, row-softmax, out = elu(alpha @ h)

v5 dataflow per (b,t), in eT = e^T [j, i] orientation so aggregation runs
as PSUM-accumulated matmuls with j as contraction dim. All ops obey the
BIR legality rules: gpsimd is SBUF-only, TT has at most one PSUM input,
stt/reciprocal are SBUF-only, matmul operand dtypes match.

    xT   : transpose-DMA loads x straight from DRAM as bf16 [d, n] --
           no PE transposes, no PSUM evacuation for x at all.
    ev   : [es; ed] = WSD.T @ xT (one matmul); two DVE tensor_scalar
           fixups build ev_rhs=[es;1], ev_lhs=[1;ed] (f32r SBUF)
    e    : rank-2 matmuls, 2-chunk halves     [j, i] PSUM (ring 2)
    z1   : Prelu halves on ACT -> z bf16 SBUF; Exp in-place (one ACT op)
    h    : xT chunks @ W (bf16) -> PSUM -> DVE evac bf16
    mask : z2 = z * M01 (0/1 bf16), split DVE/Pool by chunk
    s    : ones.T @ z2 accumulated            [1, i] PSUM
    uT   : h.T @ z2 accumulated               [f, i] PSUM
    r    : s -> SBUF (DVE), PE-transpose to columns, reciprocal [128,4]
    v    : uT -> SBUF bf16 (DVE), PE-transpose back to [i, f] chunks,
           v = u * r per chunk (DVE tensor_scalar, per-partition ptr)
    out  : elu(v) = max(v, min(exp(v)-1, 0)): ACT exp, Pool e1, Pool max
"""

import numpy as np

B, N, T, D, F = 16, 512, 12, 128, 128
NCORES = 8
B_PER_CORE = B // NCORES
NCH = N // 128  # 4 chunks of 128 nodes


def _build_program(reps=1):
    import concourse.bacc as bacc
    import concourse.tile as tile
    from concourse import mybir

    import os
    F32 = mybir.dt.float32
    F32R = mybir.dt.float32r
    BF16 = mybir.dt.bfloat16
    AF = mybir.ActivationFunctionType
    ALU = mybir.AluOpType

    nc = bacc.Bacc()

    def eng(name):
        return {"pool": nc.gpsimd, "vector": nc.vector}[name]

    # engine assignment knobs
    E_HCOPY = eng(os.environ.get("K_HCOPY", "vector"))
    E_SROW = eng(os.environ.get("K_SROW", "vector"))
    MSPLIT = int(os.environ.get("K_MSPLIT", "1"))  # chunks on DVE; rest Pool
    E_E1 = eng(os.environ.get("K_E1", "pool"))
    TAIL = os.environ.get("K_TAIL", "vone")  # mchunk|vone|dvemax
    MASKMODE = os.environ.get("K_MASKMODE", "prefill")  # prefill|mult_after
    HSTAGE = int(os.environ.get("K_HSTAGE", "1"))
    EXPSPLIT = os.environ.get("K_EXPSPLIT", "0") == "1"
    E_OMAX = eng(os.environ.get("K_OMAX", "pool"))

    x_h = nc.declare_dram_parameter("x", [B_PER_CORE, N, T, D], BF16, isOutput=False)
    wb_h = nc.declare_dram_parameter("wb", [D, F], BF16, isOutput=False)
    wsd_h = nc.declare_dram_parameter("wsd", [D, 2], BF16, isOutput=False)
    capt_h = nc.declare_dram_parameter("capt", [NCH, 128, N], BF16, isOutput=False)
    maskt_h = nc.declare_dram_parameter("maskt", [NCH, 128, N], BF16, isOutput=False)
    sel_h = nc.declare_dram_parameter("sel", [2, 2], F32, isOutput=False)
    ident_h = nc.declare_dram_parameter("ident", [128, 128], F32, isOutput=False)
    identb_h = nc.declare_dram_parameter("identb", [128, 128], BF16, isOutput=False)
    ones_h = nc.declare_dram_parameter("onescol", [128, 1], BF16, isOutput=False)
    out_h = nc.declare_dram_parameter("out", [B_PER_CORE, N, T, F], F32, isOutput=True)

    NBT = B_PER_CORE * T

    with tile.TileContext(nc) as tc:
        with (
            tc.tile_pool(name="consts", bufs=1) as consts,
            tc.tile_pool(name="xbuf", bufs=1) as xbuf,
            tc.tile_pool(name="work", bufs=int(os.environ.get("K_WORK", "5"))) as work,
            tc.tile_pool(name="zpool", bufs=int(os.environ.get("K_BIG", "5"))) as zpool,
            tc.tile_pool(name="z2pool", bufs=int(os.environ.get("K_BIG2", "5"))) as z2pool,
            tc.tile_pool(name="hpool", bufs=int(os.environ.get("K_HP", "5"))) as hpool,
            # PSUM (16KB/partition): mm{ev,h,s,v} ring3 = 6KB,
            # eadd halves [128,2,512] ring2 = 8KB, ut{uT,sc} ring1 = 2KB
            tc.tile_pool(name="mm_ps", bufs=int(os.environ.get("K_MM", "3")), space="PSUM") as mm_ps,
            tc.tile_pool(name="eadd_ps", bufs=int(os.environ.get("K_EADD", "2")), space="PSUM") as eadd_ps,
            tc.tile_pool(name="ut_ps", bufs=int(os.environ.get("K_UT", "1")), space="PSUM") as ut_psp,
        ):
            wb_sb = consts.tile([D, F], BF16)
            wsd_sb = consts.tile([D, 2], BF16)
            cap_sb = consts.tile([128, NCH, N], BF16)
            mask_sb = consts.tile([128, NCH, N], BF16)
            sel_sb = consts.tile([2, 2], F32)
            id_sb = consts.tile([128, 128], F32)
            idb_sb = consts.tile([128, 128], BF16)
            ones_sb = consts.tile([128, 1], BF16)
            nc.sync.dma_start(out=wb_sb, in_=wb_h[:, :])
            nc.sync.dma_start(out=wsd_sb, in_=wsd_h[:, :])
            if MASKMODE != "prefill":
                for c in range(NCH):
                    nc.sync.dma_start(out=cap_sb[:, c, :], in_=capt_h[c, :, :])

            nc.sync.dma_start(out=sel_sb, in_=sel_h[:, :])
            nc.sync.dma_start(out=id_sb, in_=ident_h[:, :])
            nc.sync.dma_start(out=idb_sb, in_=identb_h[:, :])
            nc.sync.dma_start(out=ones_sb, in_=ones_h[:, :])

            # ---- transpose-DMA all of x: [n, d] slices land as [d, n] bf16.
            #      Chunked [128, 128] so engines spread and bt 0 starts early.
            XCH = int(os.environ.get("K_XCH", "2"))  # chunks per x-DMA
            XPRI = int(os.environ.get("K_XPRI", "1"))  # bts loaded pre-mask
            xT_all = xbuf.tile([128, NBT, N], BF16, tag="xT")

            def load_x(k):
                b, t = divmod(k, T)
                for c0 in range(0, NCH, XCH):
                    c1 = min(c0 + XCH, NCH)
                    nc.sync.dma_start_transpose(
                        out=xT_all[:, k, c0 * 128:c1 * 128],
                        in_=x_h[b, c0 * 128:c1 * 128, t, :])

            # first iterations' x before the big mask consts, rest after
            for k in range(min(XPRI, NBT)):
                load_x(k)
            if MASKMODE == "prefill":
                for c in range(NCH):
                    nc.sync.dma_start(out=mask_sb[:, c, :], in_=maskt_h[c, :, :])
            for k in range(min(XPRI, NBT), NBT):
                load_x(k)

            # persistent ev operand rings: row1 of ev_rhs and row0 of
            # ev_lhs hold constant 1.0 (memset once); per-bt one DVE
            # evacuate + two SBUF-to-SBUF row DMAs fill in es/ed.
            EVFIX = os.environ.get("K_EVFIX", "engines")
            EVN = int(os.environ.get("K_EVN", "4"))
            ev_rhs_ring = []
            ev_lhs_ring = []
            if EVFIX == "dma":
                for i in range(EVN):
                    er = work.tile([2, N], F32R, tag="ev_rhs", name=f"evr{i}")
                    el = work.tile([2, N], F32R, tag="ev_lhs", name=f"evl{i}")
                    nc.vector.memset(er[1:2, :], 1.0)
                    nc.vector.memset(el[0:1, :], 1.0)
                    ev_rhs_ring.append(er)
                    ev_lhs_ring.append(el)

            st = [dict() for _ in range(NBT)]

            def stage1(k):
                xT = xT_all[:, k, :]
                # ev rows [es; ed] -> ev_rhs = [es; ones], ev_lhs = [ones; ed]
                ev_ps = mm_ps.tile([2, N], F32, tag="mm")
                nc.tensor.matmul(ev_ps, wsd_sb, xT, start=True, stop=True)
                if EVFIX == "dma":
                    evb_sb = work.tile([2, N], F32R, tag="evb")
                    nc.vector.tensor_copy(out=evb_sb, in_=ev_ps)
                    ev_rhs = ev_rhs_ring[k % EVN]
                    ev_lhs = ev_lhs_ring[k % EVN]
                    nc.sync.dma_start(out=ev_rhs[0:1, :], in_=evb_sb[0:1, :])
                    nc.sync.dma_start(out=ev_lhs[1:2, :], in_=evb_sb[1:2, :])
                else:
                    ev_rhs = work.tile([2, N], F32R, tag="ev_rhs")
                    ev_lhs = work.tile([2, N], F32R, tag="ev_lhs")
                    nc.vector.tensor_scalar(
                        out=ev_rhs, in0=ev_ps, scalar1=sel_sb[:, 0:1],
                        scalar2=sel_sb[:, 1:2], op0=ALU.mult, op1=ALU.add)
                    nc.vector.tensor_scalar(
                        out=ev_lhs, in0=ev_ps, scalar1=sel_sb[:, 1:2],
                        scalar2=sel_sb[:, 0:1], op0=ALU.mult, op1=ALU.add)

                # rank-2 logits by halves -> Prelu -> z bf16. In prefill
                # mode the mask bias (0/-1e9) is accumulated into PSUM by an
                # identity matmul before the rank-2, so no post-exp multiply.
                z_sb = zpool.tile([128, NCH, N], BF16, tag="z_sb")
                EW = int(os.environ.get("K_EW", "2"))  # chunks per eadd tile
                for ha in range(NCH // EW):
                    e_ps = eadd_ps.tile([128, EW, N], F32, tag="eadd")
                    for ci in range(EW):
                        c = EW * ha + ci
                        if MASKMODE == "prefill":
                            nc.tensor.matmul(
                                e_ps[:, ci, :], idb_sb, mask_sb[:, c, :],
                                start=True, stop=False)
                        nc.tensor.matmul(
                            e_ps[:, ci, :],
                            ev_lhs[:, c * 128:(c + 1) * 128],
                            ev_rhs, start=(MASKMODE != "prefill"), stop=True)
                    nc.scalar.activation(
                        z_sb[:, EW * ha:EW * (ha + 1), :], e_ps,
                        AF.Prelu, alpha=0.2)

                def do_h():
                    h_ps = mm_ps.tile([128, NCH, F], F32, tag="mm")
                    for c in range(NCH):
                        nc.tensor.matmul(
                            h_ps[:, c, :], xT[:, c * 128:(c + 1) * 128],
                            wb_sb, start=True, stop=True)
                    h_sb = hpool.tile([128, NCH, F], BF16, tag="h_sb")
                    E_HCOPY.tensor_copy(out=h_sb, in_=h_ps)
                    st[k]["h_sb"] = h_sb
                if HSTAGE == 1:
                    do_h()
                else:
                    st[k]["do_h"] = do_h

                # exp in place; mult_after additionally masks into z2
                if EXPSPLIT:
                    nc.scalar.activation(z_sb[:, 0:2, :], z_sb[:, 0:2, :], AF.Exp)
                    nc.scalar.activation(z_sb[:, 2:4, :], z_sb[:, 2:4, :], AF.Exp)
                else:
                    nc.scalar.activation(z_sb, z_sb, AF.Exp)
                if MASKMODE == "prefill":
                    z2_sb = z_sb
                else:
                    z2_sb = z2pool.tile([128, NCH, N], BF16, tag="z2_sb")
                    m = MSPLIT
                    if m > 0:
                        nc.vector.tensor_tensor(
                            out=z2_sb[:, 0:m, :], in0=z_sb[:, 0:m, :],
                            in1=cap_sb[:, 0:m, :], op=ALU.mult)
                    if m < NCH:
                        nc.gpsimd.tensor_tensor(
                            out=z2_sb[:, m:NCH, :], in0=z_sb[:, m:NCH, :],
                            in1=cap_sb[:, m:NCH, :], op=ALU.mult)
                st[k]["z2_sb"] = z2_sb

            def stage2(k):
                if HSTAGE == 2:
                    st[k]["do_h"]()
                h_sb, z2_sb = st[k]["h_sb"], st[k]["z2_sb"]
                s_ps = mm_ps.tile([1, N], F32, tag="mm")
                for c in range(NCH):
                    nc.tensor.matmul(s_ps, ones_sb, z2_sb[:, c, :],
                                     start=(c == 0), stop=(c == NCH - 1))
                uT_ps = ut_psp.tile([128, N], F32, tag="ut")
                for c in range(NCH):
                    nc.tensor.matmul(uT_ps, h_sb[:, c, :], z2_sb[:, c, :],
                                     start=(c == 0), stop=(c == NCH - 1))

                s_row = work.tile([1, N], F32, tag="s_row")
                E_SROW.tensor_copy(out=s_row, in_=s_ps)
                uT_sb = work.tile([128, N], BF16, tag="uT_sb")
                nc.vector.tensor_copy(out=uT_sb, in_=uT_ps)
                sc_ps = ut_psp.tile([128, NCH], F32, tag="ut")
                for c in range(NCH):
                    nc.tensor.transpose(
                        sc_ps[:, c:c + 1], s_row[0:1, c * 128:(c + 1) * 128],
                        id_sb[0:1, 0:1])
                sc_sb = work.tile([128, NCH], F32, tag="sc_sb")
                nc.vector.tensor_copy(out=sc_sb, in_=sc_ps)
                r_cols = work.tile([128, NCH], F32, tag="r_cols")
                nc.vector.reciprocal_approx_fast(r_cols, sc_sb)
                st[k]["uT_sb"] = uT_sb
                st[k]["r_cols"] = r_cols

            def stage3(k):
                b, t = divmod(k, T)
                uT_sb, r_cols = st[k]["uT_sb"], st[k]["r_cols"]
                u_ps = mm_ps.tile([128, NCH, F], BF16, tag="mm")
                for c in range(NCH):
                    nc.tensor.transpose(
                        u_ps[:, c, :], uT_sb[:, c * 128:(c + 1) * 128], idb_sb)
                # elu(v) = relu(v) + min(exp(v)-1, 0) = max(v, min(exp(v)-1, 0))
                if TAIL == "mchunk":
                    # m = max(u*r,0) DVE per chunk; t = exp(u*r) ACT per chunk
                    m_sb = work.tile([128, NCH, F], F32, tag="m_sb")
                    t_sb = work.tile([128, NCH, F], F32, tag="t_sb")
                    for c in range(NCH):
                        nc.vector.tensor_scalar(
                            out=m_sb[:, c, :], in0=u_ps[:, c, :],
                            scalar1=r_cols[:, c:c + 1], scalar2=0.0,
                            op0=ALU.mult, op1=ALU.max)
                        nc.scalar.activation(
                            t_sb[:, c, :], u_ps[:, c, :], AF.Exp,
                            scale=r_cols[:, c:c + 1])
                    e1_sb = work.tile([128, NCH, F], F32, tag="e1_sb")
                    E_E1.tensor_scalar(
                        out=e1_sb, in0=t_sb, scalar1=-1.0, scalar2=0.0,
                        op0=ALU.add, op1=ALU.min)
                    o_sb = work.tile([128, NCH, F], F32, tag="o_sb")
                    E_OMAX.tensor_tensor(
                        out=o_sb, in0=m_sb, in1=e1_sb, op=ALU.add)
                else:
                    # v materialized by DVE; one ACT exp; m on Pool ts;
                    # tail == "vone": o = m + e1 (Pool); "dvemax": o = DVE max
                    v_sb = work.tile([128, NCH, F], F32, tag="v_sb")
                    for c in range(NCH):
                        nc.vector.tensor_scalar(
                            out=v_sb[:, c, :], in0=u_ps[:, c, :],
                            scalar1=r_cols[:, c:c + 1], scalar2=None,
                            op0=ALU.mult)
                    t_sb = work.tile([128, NCH, F], F32, tag="t_sb")
                    nc.scalar.activation(t_sb, v_sb, AF.Exp)
                    e1_sb = work.tile([128, NCH, F], F32, tag="e1_sb")
                    E_E1.tensor_scalar(
                        out=e1_sb, in0=t_sb, scalar1=-1.0, scalar2=0.0,
                        op0=ALU.add, op1=ALU.min)
                    o_sb = work.tile([128, NCH, F], F32, tag="o_sb")
                    if TAIL == "dvemax":
                        nc.vector.tensor_tensor(
                            out=o_sb, in0=v_sb, in1=e1_sb, op=ALU.max)
                    else:
                        m_sb = work.tile([128, NCH, F], F32, tag="m_sb")
                        nc.gpsimd.tensor_scalar(
                            out=m_sb, in0=v_sb, scalar1=0.0, scalar2=None,
                            op0=ALU.max)
                        E_OMAX.tensor_tensor(
                            out=o_sb, in0=m_sb, in1=e1_sb, op=ALU.add)
                o_dst = out_h[b, :, t, :].rearrange("(c p) f -> p c f", p=128)
                nc.sync.dma_start(out=o_dst, in_=o_sb)
                st[k].clear()

            # software-pipelined emission with stage lag
            LAG = int(os.environ.get("K_LAG", "3"))

            ORDER = os.environ.get("K_ORDER", "312")

            def body(_iv=None, unroll=1):
                for k in range(NBT + 2 * LAG):
                    def s1():
                        if k < NBT:
                            stage1(k)
                    def s2():
                        if LAG <= k < NBT + LAG:
                            stage2(k - LAG)
                    def s3():
                        if k >= 2 * LAG:
                            stage3(k - 2 * LAG)
                    for ch in ORDER:
                        {"1": s1, "2": s2, "3": s3}[ch]()

            if reps == 1:
                body()
            else:
                with tc.For_i(0, reps, 1) as _iv:
                    body(_iv)

    nc.finalize()
    return nc


def prepare(x, W, a_src, a_dst, adj):
    """Build the program + per-core input maps (shared by kernel() and bench)."""
    import ml_dtypes

    bf16 = ml_dtypes.bfloat16
    x = np.ascontiguousarray(x, dtype=np.float32)
    W = np.ascontiguousarray(W, dtype=np.float32)
    a_src = np.asarray(a_src, dtype=np.float32)
    a_dst = np.asarray(a_dst, dtype=np.float32)
    adj = np.asarray(adj)

    allowed = (adj > 0) | np.eye(N, dtype=bool)               # [i, j]
    m01 = np.where(allowed, 1.0, 0.0).astype(np.float32)      # [i, j]
    capt = np.ascontiguousarray(m01.T.reshape(NCH, 128, N)).astype(bf16)
    mneg = np.where(allowed, 0.0, -1e9).astype(np.float32)
    maskt = np.ascontiguousarray(mneg.T.reshape(NCH, 128, N)).astype(bf16)
    ws = W @ a_src
    wd = W @ a_dst
    wsd = np.stack([ws, wd], axis=1).astype(bf16)
    sel = np.array([[1.0, 0.0], [0.0, 1.0]], dtype=np.float32)
    ident = np.eye(128, dtype=np.float32)
    onescol = np.ones((128, 1), dtype=np.float32).astype(bf16)

    nc = _build_program()

    in_maps = []
    for c in range(NCORES):
        in_maps.append({
            "x": np.ascontiguousarray(x[c * B_PER_CORE:(c + 1) * B_PER_CORE]).astype(bf16),
            "wb": W.astype(bf16), "wsd": wsd, "capt": capt, "sel": sel,
            "ident": ident, "identb": ident.astype(bf16), "onescol": onescol,
            "maskt": maskt,
        })
    return nc, in_maps


def kernel(x, W, a_src, a_dst, adj):
    from concourse.bass_utils import run_bass_kernel_spmd

    nc, in_maps = prepare(x, W, a_src, a_dst, adj)
    res = run_bass_kernel_spmd(nc, in_maps, list(range(NCORES)))
    out = np.concatenate([res.results[c]["out"] for c in range(NCORES)], axis=0)
    return out  # [B, N, T, F]


# revision 28
# speedup vs baseline: 1.0437x; 1.0064x over previous
"""GAT layer kernel for Trainium2 (8 NeuronCores, SPMD data-parallel over B).

Reference computation (per (b,t) slice, N=512 nodes, D=F=128):
    h = x_bt @ W
    e[i,j] = leaky_relu(e_src[i] + e_dst[j], 0.2)
    e masked by adj|I, row-softmax, out = elu(alpha @ h)

v5 dataflow per (b,t), in eT = e^T [j, i] orientation so aggregation runs
as PSUM-accumulated matmuls with j as contraction dim. All ops obey the
BIR legality rules: gpsimd is SBUF-only, TT has at most one PSUM input,
stt/reciprocal are SBUF-only, matmul operand dtypes match.

    xT   : transpose-DMA loads x straight from DRAM as bf16 [d, n] --
           no PE transposes, no PSUM evacuation for x at all.
    ev   : [es; ed] = WSD.T @ xT (one matmul); two DVE tensor_scalar
           fixups build ev_rhs=[es;1], ev_lhs=[1;ed] (f32r SBUF)
    e    : rank-2 matmuls, 2-chunk halves     [j, i] PSUM (ring 2)
    z1   : Prelu halves on ACT -> z bf16 SBUF; Exp in-place (one ACT op)
    h    : xT chunks @ W (bf16) -> PSUM -> DVE evac bf16
    mask : z2 = z * M01 (0/1 bf16), split DVE/Pool by chunk
    s    : ones.T @ z2 accumulated            [1, i] PSUM
    uT   : h.T @ z2 accumulated               [f, i] PSUM
    r    : s -> SBUF (DVE), PE-transpose to columns, reciprocal [128,4]
    v    : uT -> SBUF bf16 (DVE), PE-transpose back to [i, f] chunks,
           v = u * r per chunk (DVE tensor_scalar, per-partition ptr)
    out  : elu(v) = max(v, min(exp(v)-1, 0)): ACT exp, Pool e1, Pool max
"""

import numpy as np

B, N, T, D, F = 16, 512, 12, 128, 128
NCORES = 8
B_PER_CORE = B // NCORES
NCH = N // 128  # 4 chunks of 128 nodes


def _build_program(reps=1):
    import concourse.bacc as bacc
    import concourse.tile as tile
    from concourse import mybir

    import os
    F32 = mybir.dt.float32
    F32R = mybir.dt.float32r
    BF16 = mybir.dt.bfloat16
    AF = mybir.ActivationFunctionType
    ALU = mybir.AluOpType

    nc = bacc.Bacc()

    def eng(name):
        return {"pool": nc.gpsimd, "vector": nc.vector}[name]

    # engine assignment knobs
    E_HCOPY = eng(os.environ.get("K_HCOPY", "vector"))
    E_SROW = eng(os.environ.get("K_SROW", "vector"))
    MSPLIT = int(os.environ.get("K_MSPLIT", "1"))  # chunks on DVE; rest Pool
    E_E1 = eng(os.environ.get("K_E1", "pool"))
    TAIL = os.environ.get("K_TAIL", "vone")  # mchunk|vone|dvemax
    MASKMODE = os.environ.get("K_MASKMODE", "prefill")  # prefill|mult_after
    HSTAGE = int(os.environ.get("K_HSTAGE", "1"))
    EXPSPLIT = os.environ.get("K_EXPSPLIT", "0") == "1"
    E_OMAX = eng(os.environ.get("K_OMAX", "pool"))

    x_h = nc.declare_dram_parameter("x", [B_PER_CORE, N, T, D], BF16, isOutput=False)
    wb_h = nc.declare_dram_parameter("wb", [D, F], BF16, isOutput=False)
    wsd_h = nc.declare_dram_parameter("wsd", [D, 2], BF16, isOutput=False)
    capt_h = nc.declare_dram_parameter("capt", [NCH, 128, N], BF16, isOutput=False)
    maskt_h = nc.declare_dram_parameter("maskt", [NCH, 128, N], BF16, isOutput=False)
    sel_h = nc.declare_dram_parameter("sel", [2, 2], F32, isOutput=False)
    ident_h = nc.declare_dram_parameter("ident", [128, 128], F32, isOutput=False)
    identb_h = nc.declare_dram_parameter("identb", [128, 128], BF16, isOutput=False)
    ones_h = nc.declare_dram_parameter("onescol", [128, 1], BF16, isOutput=False)
    out_h = nc.declare_dram_parameter("out", [B_PER_CORE, N, T, F], F32, isOutput=True)

    NBT = B_PER_CORE * T

    with tile.TileContext(nc) as tc:
        with (
            tc.tile_pool(name="consts", bufs=1) as consts,
            tc.tile_pool(name="xbuf", bufs=1) as xbuf,
            tc.tile_pool(name="work", bufs=int(os.environ.get("K_WORK", "5"))) as work,
            tc.tile_pool(name="zpool", bufs=int(os.environ.get("K_BIG", "5"))) as zpool,
            tc.tile_pool(name="z2pool", bufs=int(os.environ.get("K_BIG2", "5"))) as z2pool,
            tc.tile_pool(name="hpool", bufs=int(os.environ.get("K_HP", "5"))) as hpool,
            # PSUM (16KB/partition): mm{ev,h,s,v} ring3 = 6KB,
            # eadd halves [128,2,512] ring2 = 8KB, ut{uT,sc} ring1 = 2KB
            tc.tile_pool(name="mm_ps", bufs=int(os.environ.get("K_MM", "3")), space="PSUM") as mm_ps,
            tc.tile_pool(name="eadd_ps", bufs=int(os.environ.get("K_EADD", "2")), space="PSUM") as eadd_ps,
            tc.tile_pool(name="ut_ps", bufs=int(os.environ.get("K_UT", "1")), space="PSUM") as ut_psp,
        ):
            wb_sb = consts.tile([D, F], BF16)
            wsd_sb = consts.tile([D, 2], BF16)
            cap_sb = consts.tile([128, NCH, N], BF16)
            mask_sb = consts.tile([128, NCH, N], BF16)
            sel_sb = consts.tile([2, 2], F32)
            id_sb = consts.tile([128, 128], F32)
            idb_sb = consts.tile([128, 128], BF16)
            ones_sb = consts.tile([128, 1], BF16)
            nc.sync.dma_start(out=wb_sb, in_=wb_h[:, :])
            nc.sync.dma_start(out=wsd_sb, in_=wsd_h[:, :])
            if MASKMODE != "prefill":
                for c in range(NCH):
                    nc.sync.dma_start(out=cap_sb[:, c, :], in_=capt_h[c, :, :])

            nc.sync.dma_start(out=sel_sb, in_=sel_h[:, :])
            nc.sync.dma_start(out=id_sb, in_=ident_h[:, :])
            nc.sync.dma_start(out=idb_sb, in_=identb_h[:, :])
            nc.sync.dma_start(out=ones_sb, in_=ones_h[:, :])

            # ---- transpose-DMA all of x: [n, d] slices land as [d, n] bf16.
            #      Chunked [128, 128] so engines spread and bt 0 starts early.
            XCH = int(os.environ.get("K_XCH", "4"))  # chunks per x-DMA
            XPRI = int(os.environ.get("K_XPRI", "1"))  # bts loaded pre-mask
            xT_all = xbuf.tile([128, NBT, N], BF16, tag="xT")

            def load_x(k):
                b, t = divmod(k, T)
                for c0 in range(0, NCH, XCH):
                    c1 = min(c0 + XCH, NCH)
                    nc.sync.dma_start_transpose(
                        out=xT_all[:, k, c0 * 128:c1 * 128],
                        in_=x_h[b, c0 * 128:c1 * 128, t, :])

            # first iterations' x before the big mask consts, rest after
            for k in range(min(XPRI, NBT)):
                load_x(k)
            if MASKMODE == "prefill":
                for c in range(NCH):
                    nc.sync.dma_start(out=mask_sb[:, c, :], in_=maskt_h[c, :, :])
            for k in range(min(XPRI, NBT), NBT):
                load_x(k)

            # persistent ev operand rings: row1 of ev_rhs and row0 of
            # ev_lhs hold constant 1.0 (memset once); per-bt one DVE
            # evacuate + two SBUF-to-SBUF row DMAs fill in es/ed.
            EVFIX = os.environ.get("K_EVFIX", "engines")
            EVN = int(os.environ.get("K_EVN", "4"))
            ev_rhs_ring = []
            ev_lhs_ring = []
            if EVFIX == "dma":
                for i in range(EVN):
                    er = work.tile([2, N], F32R, tag="ev_rhs", name=f"evr{i}")
                    el = work.tile([2, N], F32R, tag="ev_lhs", name=f"evl{i}")
                    nc.vector.memset(er[1:2, :], 1.0)
                    nc.vector.memset(el[0:1, :], 1.0)
                    ev_rhs_ring.append(er)
                    ev_lhs_ring.append(el)

            st = [dict() for _ in range(NBT)]

            def stage1(k):
                xT = xT_all[:, k, :]
                # ev rows [es; ed] -> ev_rhs = [es; ones], ev_lhs = [ones; ed]
                ev_ps = mm_ps.tile([2, N], F32, tag="mm")
                nc.tensor.matmul(ev_ps, wsd_sb, xT, start=True, stop=True)
                if EVFIX == "dma":
                    evb_sb = work.tile([2, N], F32R, tag="evb")
                    nc.vector.tensor_copy(out=evb_sb, in_=ev_ps)
                    ev_rhs = ev_rhs_ring[k % EVN]
                    ev_lhs = ev_lhs_ring[k % EVN]
                    nc.sync.dma_start(out=ev_rhs[0:1, :], in_=evb_sb[0:1, :])
                    nc.sync.dma_start(out=ev_lhs[1:2, :], in_=evb_sb[1:2, :])
                else:
                    ev_rhs = work.tile([2, N], F32R, tag="ev_rhs")
                    ev_lhs = work.tile([2, N], F32R, tag="ev_lhs")
                    nc.vector.tensor_scalar(
                        out=ev_rhs, in0=ev_ps, scalar1=sel_sb[:, 0:1],
                        scalar2=sel_sb[:, 1:2], op0=ALU.mult, op1=ALU.add)
                    nc.vector.tensor_scalar(
                        out=ev_lhs, in0=ev_ps, scalar1=sel_sb[:, 1:2],
                        scalar2=sel_sb[:, 0:1], op0=ALU.mult, op1=ALU.add)

                # rank-2 logits by halves -> Prelu -> z bf16. In prefill
                # mode the mask bias (0/-1e9) is accumulated into PSUM by an
                # identity matmul before the rank-2, so no post-exp multiply.
                z_sb = zpool.tile([128, NCH, N], BF16, tag="z_sb")
                EW = int(os.environ.get("K_EW", "2"))  # chunks per eadd tile
                for ha in range(NCH // EW):
                    e_ps = eadd_ps.tile([128, EW, N], F32, tag="eadd")
                    for ci in range(EW):
                        c = EW * ha + ci
                        if MASKMODE == "prefill":
                            nc.tensor.matmul(
                                e_ps[:, ci, :], idb_sb, mask_sb[:, c, :],
                                start=True, stop=False)
                        nc.tensor.matmul(
                            e_ps[:, ci, :],
                            ev_lhs[:, c * 128:(c + 1) * 128],
                            ev_rhs, start=(MASKMODE != "prefill"), stop=True)
                    nc.scalar.activation(
                        z_sb[:, EW * ha:EW * (ha + 1), :], e_ps,
                        AF.Prelu, alpha=0.2)

                def do_h():
                    h_ps = mm_ps.tile([128, NCH, F], F32, tag="mm")
                    for c in range(NCH):
                        nc.tensor.matmul(
                            h_ps[:, c, :], xT[:, c * 128:(c + 1) * 128],
                            wb_sb, start=True, stop=True)
                    h_sb = hpool.tile([128, NCH, F], BF16, tag="h_sb")
                    E_HCOPY.tensor_copy(out=h_sb, in_=h_ps)
                    st[k]["h_sb"] = h_sb
                if HSTAGE == 1:
                    do_h()
                else:
                    st[k]["do_h"] = do_h

                # exp in place; mult_after additionally masks into z2
                if EXPSPLIT:
                    nc.scalar.activation(z_sb[:, 0:2, :], z_sb[:, 0:2, :], AF.Exp)
                    nc.scalar.activation(z_sb[:, 2:4, :], z_sb[:, 2:4, :], AF.Exp)
                else:
                    nc.scalar.activation(z_sb, z_sb, AF.Exp)
                if MASKMODE == "prefill":
                    z2_sb = z_sb
                else:
                    z2_sb = z2pool.tile([128, NCH, N], BF16, tag="z2_sb")
                    m = MSPLIT
                    if m > 0:
                        nc.vector.tensor_tensor(
                            out=z2_sb[:, 0:m, :], in0=z_sb[:, 0:m, :],
                            in1=cap_sb[:, 0:m, :], op=ALU.mult)
                    if m < NCH:
                        nc.gpsimd.tensor_tensor(
                            out=z2_sb[:, m:NCH, :], in0=z_sb[:, m:NCH, :],
                            in1=cap_sb[:, m:NCH, :], op=ALU.mult)
                st[k]["z2_sb"] = z2_sb

            def stage2(k):
                if HSTAGE == 2:
                    st[k]["do_h"]()
                h_sb, z2_sb = st[k]["h_sb"], st[k]["z2_sb"]
                s_ps = mm_ps.tile([1, N], F32, tag="mm")
                for c in range(NCH):
                    nc.tensor.matmul(s_ps, ones_sb, z2_sb[:, c, :],
                                     start=(c == 0), stop=(c == NCH - 1))
                uT_ps = ut_psp.tile([128, N], F32, tag="ut")
                for c in range(NCH):
                    nc.tensor.matmul(uT_ps, h_sb[:, c, :], z2_sb[:, c, :],
                                     start=(c == 0), stop=(c == NCH - 1))

                s_row = work.tile([1, N], F32, tag="s_row")
                E_SROW.tensor_copy(out=s_row, in_=s_ps)
                uT_sb = work.tile([128, N], BF16, tag="uT_sb")
                nc.vector.tensor_copy(out=uT_sb, in_=uT_ps)
                sc_ps = ut_psp.tile([128, NCH], F32, tag="ut")
                for c in range(NCH):
                    nc.tensor.transpose(
                        sc_ps[:, c:c + 1], s_row[0:1, c * 128:(c + 1) * 128],
                        id_sb[0:1, 0:1])
                sc_sb = work.tile([128, NCH], F32, tag="sc_sb")
                nc.vector.tensor_copy(out=sc_sb, in_=sc_ps)
                r_cols = work.tile([128, NCH], F32, tag="r_cols")
                nc.vector.reciprocal_approx_fast(r_cols, sc_sb)
                st[k]["uT_sb"] = uT_sb
                st[k]["r_cols"] = r_cols

            def stage3(k):
                b, t = divmod(k, T)
                uT_sb, r_cols = st[k]["uT_sb"], st[k]["r_cols"]
                u_ps = mm_ps.tile([128, NCH, F], BF16, tag="mm")
                for c in range(NCH):
                    nc.tensor.transpose(
                        u_ps[:, c, :], uT_sb[:, c * 128:(c + 1) * 128], idb_sb)
                # elu(v) = relu(v) + min(exp(v)-1, 0) = max(v, min(exp(v)-1, 0))
                if TAIL == "mchunk":
                    # m = max(u*r,0) DVE per chunk; t = exp(u*r) ACT per chunk
                    m_sb = work.tile([128, NCH, F], F32, tag="m_sb")
                    t_sb = work.tile([128, NCH, F], F32, tag="t_sb")
                    for c in range(NCH):
                        nc.vector.tensor_scalar(
                            out=m_sb[:, c, :], in0=u_ps[:, c, :],
                            scalar1=r_cols[:, c:c + 1], scalar2=0.0,
                            op0=ALU.mult, op1=ALU.max)
                        nc.scalar.activation(
                            t_sb[:, c, :], u_ps[:, c, :], AF.Exp,
                            scale=r_cols[:, c:c + 1])
                    e1_sb = work.tile([128, NCH, F], F32, tag="e1_sb")
                    E_E1.tensor_scalar(
                        out=e1_sb, in0=t_sb, scalar1=-1.0, scalar2=0.0,
                        op0=ALU.add, op1=ALU.min)
                    o_sb = work.tile([128, NCH, F], F32, tag="o_sb")
                    E_OMAX.tensor_tensor(
                        out=o_sb, in0=m_sb, in1=e1_sb, op=ALU.add)
                else:
                    # v materialized by DVE; one ACT exp; m on Pool ts;
                    # tail == "vone": o = m + e1 (Pool); "dvemax": o = DVE max
                    v_sb = work.tile([128, NCH, F], F32, tag="v_sb")
                    for c in range(NCH):
                        nc.vector.tensor_scalar(
                            out=v_sb[:, c, :], in0=u_ps[:, c, :],
                            scalar1=r_cols[:, c:c + 1], scalar2=None,
                            op0=ALU.mult)
                    t_sb = work.tile([128, NCH, F], F32, tag="t_sb")
                    nc.scalar.activation(t_sb, v_sb, AF.Exp)
                    e1_sb = work.tile([128, NCH, F], F32, tag="e1_sb")
                    E_E1.tensor_scalar(
                        out=e1_sb, in0=t_sb, scalar1=-1.0, scalar2=0.0,
                        op0=ALU.add, op1=ALU.min)
                    o_sb = work.tile([128, NCH, F], F32, tag="o_sb")
                    if TAIL == "dvemax":
                        nc.vector.tensor_tensor(
                            out=o_sb, in0=v_sb, in1=e1_sb, op=ALU.max)
                    else:
                        m_sb = work.tile([128, NCH, F], F32, tag="m_sb")
                        nc.gpsimd.tensor_scalar(
                            out=m_sb, in0=v_sb, scalar1=0.0, scalar2=None,
                            op0=ALU.max)
                        E_OMAX.tensor_tensor(
                            out=o_sb, in0=m_sb, in1=e1_sb, op=ALU.add)
                o_dst = out_h[b, :, t, :].rearrange("(c p) f -> p c f", p=128)
                nc.sync.dma_start(out=o_dst, in_=o_sb)
                st[k].clear()

            # software-pipelined emission with stage lag
            LAG = int(os.environ.get("K_LAG", "3"))

            ORDER = os.environ.get("K_ORDER", "312")

            def body(_iv=None, unroll=1):
                for k in range(NBT + 2 * LAG):
                    def s1():
                        if k < NBT:
                            stage1(k)
                    def s2():
                        if LAG <= k < NBT + LAG:
                            stage2(k - LAG)
                    def s3():
                        if k >= 2 * LAG:
                            stage3(k - 2 * LAG)
                    for ch in ORDER:
                        {"1": s1, "2": s2, "3": s3}[ch]()

            if reps == 1:
                body()
            else:
                with tc.For_i(0, reps, 1) as _iv:
                    body(_iv)

    nc.finalize()
    return nc


def prepare(x, W, a_src, a_dst, adj):
    """Build the program + per-core input maps (shared by kernel() and bench)."""
    import ml_dtypes

    bf16 = ml_dtypes.bfloat16
    x = np.ascontiguousarray(x, dtype=np.float32)
    W = np.ascontiguousarray(W, dtype=np.float32)
    a_src = np.asarray(a_src, dtype=np.float32)
    a_dst = np.asarray(a_dst, dtype=np.float32)
    adj = np.asarray(adj)

    allowed = (adj > 0) | np.eye(N, dtype=bool)               # [i, j]
    m01 = np.where(allowed, 1.0, 0.0).astype(np.float32)      # [i, j]
    capt = np.ascontiguousarray(m01.T.reshape(NCH, 128, N)).astype(bf16)
    mneg = np.where(allowed, 0.0, -1e9).astype(np.float32)
    maskt = np.ascontiguousarray(mneg.T.reshape(NCH, 128, N)).astype(bf16)
    ws = W @ a_src
    wd = W @ a_dst
    wsd = np.stack([ws, wd], axis=1).astype(bf16)
    sel = np.array([[1.0, 0.0], [0.0, 1.0]], dtype=np.float32)
    ident = np.eye(128, dtype=np.float32)
    onescol = np.ones((128, 1), dtype=np.float32).astype(bf16)

    nc = _build_program()

    in_maps = []
    for c in range(NCORES):
        in_maps.append({
            "x": np.ascontiguousarray(x[c * B_PER_CORE:(c + 1) * B_PER_CORE]).astype(bf16),
            "wb": W.astype(bf16), "wsd": wsd, "capt": capt, "sel": sel,
            "ident": ident, "identb": ident.astype(bf16), "onescol": onescol,
            "maskt": maskt,
        })
    return nc, in_maps


def kernel(x, W, a_src, a_dst, adj):
    from concourse.bass_utils import run_bass_kernel_spmd

    nc, in_maps = prepare(x, W, a_src, a_dst, adj)
    res = run_bass_kernel_spmd(nc, in_maps, list(range(NCORES)))
    out = np.concatenate([res.results[c]["out"] for c in range(NCORES)], axis=0)
    return out  # [B, N, T, F]


# revision 34
# speedup vs baseline: 1.1009x; 1.0548x over previous
"""GAT layer kernel for Trainium2 (8 NeuronCores, SPMD data-parallel over B).

Reference computation (per (b,t) slice, N=512 nodes, D=F=128):
    h = x_bt @ W
    e[i,j] = leaky_relu(e_src[i] + e_dst[j], 0.2)
    e masked by adj|I, row-softmax, out = elu(alpha @ h)

v5 dataflow per (b,t), in eT = e^T [j, i] orientation so aggregation runs
as PSUM-accumulated matmuls with j as contraction dim. All ops obey the
BIR legality rules: gpsimd is SBUF-only, TT has at most one PSUM input,
stt/reciprocal are SBUF-only, matmul operand dtypes match.

    xT   : transpose-DMA loads x straight from DRAM as bf16 [d, n] --
           no PE transposes, no PSUM evacuation for x at all.
    ev   : [es; ed] = WSD.T @ xT (one matmul); two DVE tensor_scalar
           fixups build ev_rhs=[es;1], ev_lhs=[1;ed] (f32r SBUF)
    e    : rank-2 matmuls, 2-chunk halves     [j, i] PSUM (ring 2)
    z1   : Prelu halves on ACT -> z bf16 SBUF; Exp in-place (one ACT op)
    h    : xT chunks @ W (bf16) -> PSUM -> DVE evac bf16
    mask : z2 = z * M01 (0/1 bf16), split DVE/Pool by chunk
    s    : ones.T @ z2 accumulated            [1, i] PSUM
    uT   : h.T @ z2 accumulated               [f, i] PSUM
    r    : s -> SBUF (DVE), PE-transpose to columns, reciprocal [128,4]
    v    : uT -> SBUF bf16 (DVE), PE-transpose back to [i, f] chunks,
           v = u * r per chunk (DVE tensor_scalar, per-partition ptr)
    out  : elu(v) = max(v, min(exp(v)-1, 0)): ACT exp, Pool e1, Pool max
"""

import numpy as np

B, N, T, D, F = 16, 512, 12, 128, 128
NCORES = 8
B_PER_CORE = B // NCORES
NCH = N // 128  # 4 chunks of 128 nodes


def _build_program(reps=1):
    import concourse.bacc as bacc
    import concourse.tile as tile
    from concourse import mybir

    import os
    F32 = mybir.dt.float32
    F32R = mybir.dt.float32r
    BF16 = mybir.dt.bfloat16
    AF = mybir.ActivationFunctionType
    ALU = mybir.AluOpType

    nc = bacc.Bacc()

    def eng(name):
        return {"pool": nc.gpsimd, "vector": nc.vector}[name]

    # engine assignment knobs
    E_HCOPY = eng(os.environ.get("K_HCOPY", "vector"))
    E_SROW = eng(os.environ.get("K_SROW", "vector"))
    MSPLIT = int(os.environ.get("K_MSPLIT", "1"))  # chunks on DVE; rest Pool
    E_E1 = eng(os.environ.get("K_E1", "pool"))
    TAIL = os.environ.get("K_TAIL", "vone")  # mchunk|vone|dvemax
    MASKMODE = os.environ.get("K_MASKMODE", "prefill")  # prefill|mult_after
    HSTAGE = int(os.environ.get("K_HSTAGE", "1"))
    EXPSPLIT = os.environ.get("K_EXPSPLIT", "0") == "1"
    DRAINK = int(os.environ.get("K_DRAINK", "3"))
    E_OMAX = eng(os.environ.get("K_OMAX", "pool"))

    x_h = nc.declare_dram_parameter("x", [B_PER_CORE, N, T, D], BF16, isOutput=False)
    wb_h = nc.declare_dram_parameter("wb", [D, F], BF16, isOutput=False)
    wsd_h = nc.declare_dram_parameter("wsd", [D, 2], BF16, isOutput=False)
    capt_h = nc.declare_dram_parameter("capt", [NCH, 128, N], BF16, isOutput=False)
    maskt_h = nc.declare_dram_parameter("maskt", [NCH, 128, N], BF16, isOutput=False)
    sel_h = nc.declare_dram_parameter("sel", [2, 2], F32, isOutput=False)
    ident_h = nc.declare_dram_parameter("ident", [128, 128], F32, isOutput=False)
    identb_h = nc.declare_dram_parameter("identb", [128, 128], BF16, isOutput=False)
    ones_h = nc.declare_dram_parameter("onescol", [128, 1], BF16, isOutput=False)
    out_h = nc.declare_dram_parameter("out", [B_PER_CORE, N, T, F], F32, isOutput=True)

    NBT = B_PER_CORE * T

    with tile.TileContext(nc) as tc:
        with (
            tc.tile_pool(name="consts", bufs=1) as consts,
            tc.tile_pool(name="xbuf", bufs=1) as xbuf,
            tc.tile_pool(name="work", bufs=int(os.environ.get("K_WORK", "5"))) as work,
            tc.tile_pool(name="zpool", bufs=int(os.environ.get("K_BIG", "5"))) as zpool,
            tc.tile_pool(name="z2pool", bufs=int(os.environ.get("K_BIG2", "5"))) as z2pool,
            tc.tile_pool(name="hpool", bufs=int(os.environ.get("K_HP", "5"))) as hpool,
            # PSUM (16KB/partition): mm{ev,h,s,v} ring3 = 6KB,
            # eadd halves [128,2,512] ring2 = 8KB, ut{uT,sc} ring1 = 2KB
            tc.tile_pool(name="mm_ps", bufs=int(os.environ.get("K_MM", "3")), space="PSUM") as mm_ps,
            tc.tile_pool(name="eadd_ps", bufs=int(os.environ.get("K_EADD", "2")), space="PSUM") as eadd_ps,
            tc.tile_pool(name="ut_ps", bufs=int(os.environ.get("K_UT", "1")), space="PSUM") as ut_psp,
        ):
            wb_sb = consts.tile([D, F], BF16)
            wsd_sb = consts.tile([D, 2], BF16)
            cap_sb = consts.tile([128, NCH, N], BF16)
            mask_sb = consts.tile([128, NCH, N], BF16)
            sel_sb = consts.tile([2, 2], F32)
            id_sb = consts.tile([128, 128], F32)
            idb_sb = consts.tile([128, 128], BF16)
            ones_sb = consts.tile([128, 1], BF16)

            # ---- transpose-DMA all of x: [n, d] slices land as [d, n] bf16.
            #      Chunked [128, 128] so engines spread and bt 0 starts early.
            XCH = int(os.environ.get("K_XCH", "4"))  # chunks per x-DMA
            XPRI = int(os.environ.get("K_XPRI", "2"))  # bts loaded pre-mask
            xT_all = xbuf.tile([128, NBT, N], BF16, tag="xT")

            def load_x(k):
                b, t = divmod(k, T)
                for c0 in range(0, NCH, XCH):
                    c1 = min(c0 + XCH, NCH)
                    nc.sync.dma_start_transpose(
                        out=xT_all[:, k, c0 * 128:c1 * 128],
                        in_=x_h[b, c0 * 128:c1 * 128, t, :])

            # DMA order follows first use: x(0), ev/fixup consts, mask
            # (prefill bias), projection consts, then the remaining x.
            cq = nc.sync
            for k in range(min(XPRI, NBT)):
                load_x(k)
            cq.dma_start(out=wsd_sb, in_=wsd_h[:, :])
            cq.dma_start(out=sel_sb, in_=sel_h[:, :])
            cq.dma_start(out=idb_sb, in_=identb_h[:, :])
            if MASKMODE == "prefill":
                for c in range(NCH):
                    cq.dma_start(out=mask_sb[:, c, :], in_=maskt_h[c, :, :])
            else:
                for c in range(NCH):
                    cq.dma_start(out=cap_sb[:, c, :], in_=capt_h[c, :, :])
            cq.dma_start(out=wb_sb, in_=wb_h[:, :])
            cq.dma_start(out=ones_sb, in_=ones_h[:, :])
            cq.dma_start(out=id_sb, in_=ident_h[:, :])
            for k in range(min(XPRI, NBT), NBT):
                load_x(k)

            # persistent ev operand rings: row1 of ev_rhs and row0 of
            # ev_lhs hold constant 1.0 (memset once); per-bt one DVE
            # evacuate + two SBUF-to-SBUF row DMAs fill in es/ed.
            EVFIX = os.environ.get("K_EVFIX", "engines")
            EVN = int(os.environ.get("K_EVN", "4"))
            ev_rhs_ring = []
            ev_lhs_ring = []
            if EVFIX == "dma":
                for i in range(EVN):
                    er = work.tile([2, N], F32R, tag="ev_rhs", name=f"evr{i}")
                    el = work.tile([2, N], F32R, tag="ev_lhs", name=f"evl{i}")
                    nc.vector.memset(er[1:2, :], 1.0)
                    nc.vector.memset(el[0:1, :], 1.0)
                    ev_rhs_ring.append(er)
                    ev_lhs_ring.append(el)

            st = [dict() for _ in range(NBT)]

            def stage1(k):
                xT = xT_all[:, k, :]
                # ev rows [es; ed] -> ev_rhs = [es; ones], ev_lhs = [ones; ed]
                ev_ps = mm_ps.tile([2, N], F32, tag="mm")
                nc.tensor.matmul(ev_ps, wsd_sb, xT, start=True, stop=True)
                if EVFIX == "dma":
                    evb_sb = work.tile([2, N], F32R, tag="evb")
                    nc.vector.tensor_copy(out=evb_sb, in_=ev_ps)
                    ev_rhs = ev_rhs_ring[k % EVN]
                    ev_lhs = ev_lhs_ring[k % EVN]
                    nc.sync.dma_start(out=ev_rhs[0:1, :], in_=evb_sb[0:1, :])
                    nc.sync.dma_start(out=ev_lhs[1:2, :], in_=evb_sb[1:2, :])
                else:
                    ev_rhs = work.tile([2, N], F32R, tag="ev_rhs")
                    ev_lhs = work.tile([2, N], F32R, tag="ev_lhs")
                    nc.vector.tensor_scalar(
                        out=ev_rhs, in0=ev_ps, scalar1=sel_sb[:, 0:1],
                        scalar2=sel_sb[:, 1:2], op0=ALU.mult, op1=ALU.add)
                    nc.vector.tensor_scalar(
                        out=ev_lhs, in0=ev_ps, scalar1=sel_sb[:, 1:2],
                        scalar2=sel_sb[:, 0:1], op0=ALU.mult, op1=ALU.add)

                # rank-2 logits by halves -> Prelu -> z bf16. In prefill
                # mode the mask bias (0/-1e9) is accumulated into PSUM by an
                # identity matmul before the rank-2, so no post-exp multiply.
                z_sb = zpool.tile([128, NCH, N], BF16, tag="z_sb")
                EW = int(os.environ.get("K_EW", "2"))  # chunks per eadd tile
                for ha in range(NCH // EW):
                    e_ps = eadd_ps.tile([128, EW, N], F32, tag="eadd")
                    for ci in range(EW):
                        c = EW * ha + ci
                        if MASKMODE == "prefill":
                            nc.tensor.matmul(
                                e_ps[:, ci, :], idb_sb, mask_sb[:, c, :],
                                start=True, stop=False)
                        nc.tensor.matmul(
                            e_ps[:, ci, :],
                            ev_lhs[:, c * 128:(c + 1) * 128],
                            ev_rhs, start=(MASKMODE != "prefill"), stop=True)
                    nc.scalar.activation(
                        z_sb[:, EW * ha:EW * (ha + 1), :], e_ps,
                        AF.Prelu, alpha=0.2)

                def do_h():
                    h_ps = mm_ps.tile([128, NCH, F], F32, tag="mm")
                    for c in range(NCH):
                        nc.tensor.matmul(
                            h_ps[:, c, :], xT[:, c * 128:(c + 1) * 128],
                            wb_sb, start=True, stop=True)
                    h_sb = hpool.tile([128, NCH, F], BF16, tag="h_sb")
                    E_HCOPY.tensor_copy(out=h_sb, in_=h_ps)
                    st[k]["h_sb"] = h_sb
                if HSTAGE == 1:
                    do_h()
                else:
                    st[k]["do_h"] = do_h

                # exp in place; mult_after additionally masks into z2
                if EXPSPLIT:
                    nc.scalar.activation(z_sb[:, 0:2, :], z_sb[:, 0:2, :], AF.Exp)
                    nc.scalar.activation(z_sb[:, 2:4, :], z_sb[:, 2:4, :], AF.Exp)
                else:
                    nc.scalar.activation(z_sb, z_sb, AF.Exp)
                if MASKMODE == "prefill":
                    z2_sb = z_sb
                else:
                    z2_sb = z2pool.tile([128, NCH, N], BF16, tag="z2_sb")
                    m = MSPLIT
                    if m > 0:
                        nc.vector.tensor_tensor(
                            out=z2_sb[:, 0:m, :], in0=z_sb[:, 0:m, :],
                            in1=cap_sb[:, 0:m, :], op=ALU.mult)
                    if m < NCH:
                        nc.gpsimd.tensor_tensor(
                            out=z2_sb[:, m:NCH, :], in0=z_sb[:, m:NCH, :],
                            in1=cap_sb[:, m:NCH, :], op=ALU.mult)
                st[k]["z2_sb"] = z2_sb

            def stage2(k):
                if HSTAGE == 2:
                    st[k]["do_h"]()
                h_sb, z2_sb = st[k]["h_sb"], st[k]["z2_sb"]
                s_ps = mm_ps.tile([1, N], F32, tag="mm")
                for c in range(NCH):
                    nc.tensor.matmul(s_ps, ones_sb, z2_sb[:, c, :],
                                     start=(c == 0), stop=(c == NCH - 1))
                uT_ps = ut_psp.tile([128, N], F32, tag="ut")
                for c in range(NCH):
                    nc.tensor.matmul(uT_ps, h_sb[:, c, :], z2_sb[:, c, :],
                                     start=(c == 0), stop=(c == NCH - 1))

                s_row = work.tile([1, N], F32, tag="s_row")
                E_SROW.tensor_copy(out=s_row, in_=s_ps)
                uT_sb = work.tile([128, N], BF16, tag="uT_sb")
                nc.vector.tensor_copy(out=uT_sb, in_=uT_ps)
                sc_ps = ut_psp.tile([128, NCH], F32, tag="ut")
                for c in range(NCH):
                    nc.tensor.transpose(
                        sc_ps[:, c:c + 1], s_row[0:1, c * 128:(c + 1) * 128],
                        id_sb[0:1, 0:1])
                sc_sb = work.tile([128, NCH], F32, tag="sc_sb")
                nc.vector.tensor_copy(out=sc_sb, in_=sc_ps)
                r_cols = work.tile([128, NCH], F32, tag="r_cols")
                nc.vector.reciprocal_approx_fast(r_cols, sc_sb)
                st[k]["uT_sb"] = uT_sb
                st[k]["r_cols"] = r_cols

            def stage3(k):
                b, t = divmod(k, T)
                uT_sb, r_cols = st[k]["uT_sb"], st[k]["r_cols"]
                u_ps = mm_ps.tile([128, NCH, F], BF16, tag="mm")
                for c in range(NCH):
                    nc.tensor.transpose(
                        u_ps[:, c, :], uT_sb[:, c * 128:(c + 1) * 128], idb_sb)
                # elu(v) = relu(v) + min(exp(v)-1, 0) = max(v, min(exp(v)-1, 0))
                if TAIL == "mchunk":
                    # m = max(u*r,0) DVE per chunk; t = exp(u*r) ACT per chunk
                    m_sb = work.tile([128, NCH, F], F32, tag="m_sb")
                    t_sb = work.tile([128, NCH, F], F32, tag="t_sb")
                    for c in range(NCH):
                        nc.vector.tensor_scalar(
                            out=m_sb[:, c, :], in0=u_ps[:, c, :],
                            scalar1=r_cols[:, c:c + 1], scalar2=0.0,
                            op0=ALU.mult, op1=ALU.max)
                        nc.scalar.activation(
                            t_sb[:, c, :], u_ps[:, c, :], AF.Exp,
                            scale=r_cols[:, c:c + 1])
                    e1_sb = work.tile([128, NCH, F], F32, tag="e1_sb")
                    E_E1.tensor_scalar(
                        out=e1_sb, in0=t_sb, scalar1=-1.0, scalar2=0.0,
                        op0=ALU.add, op1=ALU.min)
                    o_sb = work.tile([128, NCH, F], F32, tag="o_sb")
                    E_OMAX.tensor_tensor(
                        out=o_sb, in0=m_sb, in1=e1_sb, op=ALU.add)
                else:
                    # v materialized by DVE; one ACT exp; m on Pool ts;
                    # tail == "vone": o = m + e1 (Pool); "dvemax": o = DVE max
                    v_sb = work.tile([128, NCH, F], F32, tag="v_sb")
                    for c in range(NCH):
                        nc.vector.tensor_scalar(
                            out=v_sb[:, c, :], in0=u_ps[:, c, :],
                            scalar1=r_cols[:, c:c + 1], scalar2=None,
                            op0=ALU.mult)
                    t_sb = work.tile([128, NCH, F], F32, tag="t_sb")
                    nc.scalar.activation(t_sb, v_sb, AF.Exp)
                    drain = k >= NBT - DRAINK
                    e1_sb = work.tile([128, NCH, F], F32, tag="e1_sb")
                    (nc.vector if drain else E_E1).tensor_scalar(
                        out=e1_sb, in0=t_sb, scalar1=-1.0, scalar2=0.0,
                        op0=ALU.add, op1=ALU.min)
                    o_sb = work.tile([128, NCH, F], F32, tag="o_sb")
                    if TAIL == "dvemax" or drain:
                        nc.vector.tensor_tensor(
                            out=o_sb, in0=v_sb, in1=e1_sb, op=ALU.max)
                    else:
                        m_sb = work.tile([128, NCH, F], F32, tag="m_sb")
                        nc.gpsimd.tensor_scalar(
                            out=m_sb, in0=v_sb, scalar1=0.0, scalar2=None,
                            op0=ALU.max)
                        E_OMAX.tensor_tensor(
                            out=o_sb, in0=m_sb, in1=e1_sb, op=ALU.add)
                o_dst = out_h[b, :, t, :].rearrange("(c p) f -> p c f", p=128)
                nc.sync.dma_start(out=o_dst, in_=o_sb)
                st[k].clear()

            # software-pipelined emission with stage lag
            LAG = int(os.environ.get("K_LAG", "3"))

            ORDER = os.environ.get("K_ORDER", "312")

            def body(_iv=None, unroll=1):
                for k in range(NBT + 2 * LAG):
                    def s1():
                        if k < NBT:
                            stage1(k)
                    def s2():
                        if LAG <= k < NBT + LAG:
                            stage2(k - LAG)
                    def s3():
                        if k >= 2 * LAG:
                            stage3(k - 2 * LAG)
                    for ch in ORDER:
                        {"1": s1, "2": s2, "3": s3}[ch]()

            if reps == 1:
                body()
            else:
                with tc.For_i(0, reps, 1) as _iv:
                    body(_iv)

    nc.finalize()
    return nc


def prepare(x, W, a_src, a_dst, adj):
    """Build the program + per-core input maps (shared by kernel() and bench)."""
    import ml_dtypes

    bf16 = ml_dtypes.bfloat16
    x = np.ascontiguousarray(x, dtype=np.float32)
    W = np.ascontiguousarray(W, dtype=np.float32)
    a_src = np.asarray(a_src, dtype=np.float32)
    a_dst = np.asarray(a_dst, dtype=np.float32)
    adj = np.asarray(adj)

    allowed = (adj > 0) | np.eye(N, dtype=bool)               # [i, j]
    m01 = np.where(allowed, 1.0, 0.0).astype(np.float32)      # [i, j]
    capt = np.ascontiguousarray(m01.T.reshape(NCH, 128, N)).astype(bf16)
    mneg = np.where(allowed, 0.0, -1e9).astype(np.float32)
    maskt = np.ascontiguousarray(mneg.T.reshape(NCH, 128, N)).astype(bf16)
    ws = W @ a_src
    wd = W @ a_dst
    wsd = np.stack([ws, wd], axis=1).astype(bf16)
    sel = np.array([[1.0, 0.0], [0.0, 1.0]], dtype=np.float32)
    ident = np.eye(128, dtype=np.float32)
    onescol = np.ones((128, 1), dtype=np.float32).astype(bf16)

    nc = _build_program()

    in_maps = []
    for c in range(NCORES):
        in_maps.append({
            "x": np.ascontiguousarray(x[c * B_PER_CORE:(c + 1) * B_PER_CORE]).astype(bf16),
            "wb": W.astype(bf16), "wsd": wsd, "capt": capt, "sel": sel,
            "ident": ident, "identb": ident.astype(bf16), "onescol": onescol,
            "maskt": maskt,
        })
    return nc, in_maps


def kernel(x, W, a_src, a_dst, adj):
    from concourse.bass_utils import run_bass_kernel_spmd

    nc, in_maps = prepare(x, W, a_src, a_dst, adj)
    res = run_bass_kernel_spmd(nc, in_maps, list(range(NCORES)))
    out = np.concatenate([res.results[c]["out"] for c in range(NCORES)], axis=0)
    return out  # [B, N, T, F]
